# revision 1
# baseline (speedup 1.0000x reference)
"""Batched KNN (K=32) on 8 Trainium2 NeuronCores.

Exploits that `batch` is sorted: the 8 batch groups are contiguous blocks of
~1024 points, and KNN only needs within-block distances.  Core b gets block b:
it computes v = -d2 = 2*x@x.T - sq_i - sq_j for its [nb, nb] block (PE matmul
+ ACT bias + POOL broadcast add), masks the diagonal via gpsimd affine_select,
and extracts the top-32 (largest v = smallest d2) per row with 4 rounds of
DVE max8 / max_index / match_replace.
"""

import os
import sys

import numpy as np

for _p in ("/opt/trn_rl_repo", "/root/.axon_site/_ro/trn_rl_repo"):
    if os.path.isdir(_p) and _p not in sys.path:
        sys.path.append(_p)

K = 32
BIG = 1e30
N_CORES = 8

LAST_EXEC_NS = None

_NC_CACHE = {}


def _build_nc(W, T, D):
    import concourse.bass as bass
    from concourse import bacc, mybir
    from concourse.tile import TileContext

    f32 = mybir.dt.float32
    u32 = mybir.dt.uint32
    KC = D // 128
    assert D % 128 == 0

    P = T * 128
    nc = bacc.Bacc(None, target_bir_lowering=False)
    xt_d = nc.dram_tensor("xt", [D, W], f32, kind="ExternalInput")
    nsqc_d = nc.dram_tensor("nsqc", [128, T], f32, kind="ExternalInput")
    sqjn_d = nc.dram_tensor("sqjn", [1, W], f32, kind="ExternalInput")
    od_d = nc.dram_tensor("od", [P, K], f32, kind="ExternalOutput")
    oi_d = nc.dram_tensor("oi", [P, K], u32, kind="ExternalOutput")

    with TileContext(nc) as tc:
        with tc.tile_pool(name="const", bufs=1) as cpool, \
             tc.tile_pool(name="work", bufs=3) as wpool, \
             tc.tile_pool(name="outp", bufs=3) as opool, \
             tc.tile_pool(name="psum", bufs=2, space="PSUM") as ppool:
            xt_sb = []
            for k in range(KC):
                xk = cpool.tile([128, W], f32, tag=f"xt{k}")
                for c0 in range(0, W, 512):
                    cn = min(512, W - c0)
                    nc.sync.dma_start(
                        xk[:, c0:c0 + cn],
                        xt_d[k * 128:(k + 1) * 128, c0:c0 + cn])
                xt_sb.append(xk)
            nsqc_sb = cpool.tile([128, T], f32, tag="nsqc")
            nc.sync.dma_start(nsqc_sb[:, :], nsqc_d[:, :])
            sqjn_sb = cpool.tile([128, W], f32, tag="sqjn")
            nc.sync.dma_start(
                sqjn_sb[:, :], sqjn_d[0:1, :].to_broadcast((128, W)))

            for t in range(T):
                q0 = t * 128
                m = min(128, W - q0)
                ps = ppool.tile([128, W], f32, tag="ps")
                for k in range(KC):
                    for c0 in range(0, W, 512):
                        cn = min(512, W - c0)
                        nc.tensor.matmul(
                            ps[:m, c0:c0 + cn],
                            xt_sb[k][:, q0:q0 + m],
                            xt_sb[k][:, c0:c0 + cn],
                            start=(k == 0),
                            stop=(k == KC - 1),
                        )
                v = wpool.tile([128, W], f32, tag="v")
                # v = 2*dot - sq_i  (bias is per-partition -sq_i)
                nc.scalar.activation(
                    v[:m, :], ps[:m, :],
                    mybir.ActivationFunctionType.Identity,
                    bias=nsqc_sb[:m, t:t + 1], scale=2.0,
                )
                # v += -sq_j  (pad cols carry -BIG here)
                nc.gpsimd.tensor_add(v[:m, :], v[:m, :], sqjn_sb[:m, :])
                # diagonal (self) -> -BIG: keep where (q0 + p - c) != 0
                nc.gpsimd.affine_select(
                    out=v[:m, :], in_=v[:m, :],
                    compare_op=mybir.AluOpType.not_equal,
                    fill=-BIG, base=q0,
                    pattern=[[-1, W]], channel_multiplier=1,
                )
                vals = opool.tile([128, K], f32, tag="vals")
                inds = opool.tile([128, K], u32, tag="inds")
                for r in range(K // 8):
                    sl = slice(8 * r, 8 * r + 8)
                    nc.vector.max(out=vals[:m, sl], in_=v[:m, :])
                    nc.vector.max_index(
                        out=inds[:m, sl], in_max=vals[:m, sl], in_values=v[:m, :])
                    if r < K // 8 - 1:
                        nc.vector.match_replace(
                            out=v[:m, :], in_to_replace=vals[:m, sl],
                            in_values=v[:m, :], imm_value=-BIG)
                d2o = opool.tile([128, K], f32, tag="d2o")
                nc.scalar.activation(
                    d2o[:m, :], vals[:m, :],
                    mybir.ActivationFunctionType.Identity, scale=-1.0)
                nc.sync.dma_start(od_d[q0:q0 + m, :], d2o[:m, :])
                nc.sync.dma_start(oi_d[q0:q0 + m, :], inds[:m, :])
    nc.finalize()
    return nc


def kernel(x, batch):
    global LAST_EXEC_NS
    from concourse.bass_utils import run_bass_kernel_spmd

    x = np.ascontiguousarray(np.asarray(x), dtype=np.float32)
    b = np.asarray(batch)
    N, D = x.shape
    bounds = np.searchsorted(b, np.arange(N_CORES + 1))
    sizes = np.diff(bounds)
    W = max(128, int(-(-sizes.max() // 8)) * 8)
    T = max(1, int(-(-sizes.max() // 128)))

    key = (W, T, D)
    if key not in _NC_CACHE:
        _NC_CACHE[key] = _build_nc(W, T, D)
    nc = _NC_CACHE[key]

    in_maps = []
    for c in range(N_CORES):
        s, e = int(bounds[c]), int(bounds[c + 1])
        n = e - s
        xc = x[s:e]
        xt = np.zeros((D, W), np.float32)
        xt[:, :n] = xc.T
        sq = np.einsum("ij,ij->i", xc, xc, dtype=np.float32)
        sq_pad = np.zeros(T * 128, np.float32)
        sq_pad[:n] = sq
        nsqc = np.ascontiguousarray((-sq_pad).reshape(T, 128).T)
        row = np.full(W, -BIG, np.float32)
        row[:n] = -sq
        sqjn = row.reshape(1, W)
        in_maps.append({"xt": xt, "nsqc": nsqc, "sqjn": sqjn})

    trace = os.environ.get("KNN_TRACE", "0") == "1"
    res = run_bass_kernel_spmd(
        nc, in_maps, core_ids=list(range(N_CORES)), trace=trace)
    LAST_EXEC_NS = res.exec_time_ns

    out_d = np.empty((N, K), np.float32)
    out_i = np.empty((N, K), np.int32)
    for c in range(N_CORES):
        s, e = int(bounds[c]), int(bounds[c + 1])
        n = e - s
        if n == 0:
            continue
        out_d[s:e] = res.results[c]["od"][:n]
        out_i[s:e] = res.results[c]["oi"][:n].astype(np.int64) + s
    return out_d, out_i

